# revision 1
# baseline (speedup 1.0000x reference)
"""Trainium2 Bass kernel for nn_AttentionMask_13048110645633.

Math: for key (4,32,64,64) and query (4,512), with s = key.reshape(B,J)
and q = query, the reference computes per element

    ctx[b,j] = sum_k q[b,k]*exp(s[b,j]*q[b,k]) / sum_k exp(s[b,j]*q[b,k])
    out[b,j] = s[b,j] * sigmoid(ctx[b,j])

i.e. out = s * g_b(s) where g_b is a smooth scalar function determined by
q[b].  Sharding: data-parallel over B (4 batches x 2 half-slabs = 8 cores).

Fast variant, per core (one (128,512) tile = half a batch):
  1. broadcast q to all partitions with a C=2 PE matmul over an exact
     fp16 (qhi, qlo) pair (no 256KB DMA),
  2. evaluate g_b exactly at 128 fixed Chebyshev nodes: ACT exp with
     per-partition scale + fused accumulate, one fused DVE multiply-reduce,
     reciprocal, tanh -> 128 gate values,
  3. contract with a constant least-squares matrix on the PE to get the
     Chebyshev coefficients of g_b in t = tanh(a*s), replicated on all
     partitions,
  4. evaluate with a Clenshaw recurrence on the DVE over the whole tile
     (high-order steps in fp16 at 2 elem/cycle, low-order in fp32),
  5. out = s * g.

The direct variant brute-forces the (J,K) slab; used for cross-checking.
"""

import os
import numpy as np

B, J, K = 4, 131072, 512
P, F = 128, 512  # per-core tile (P*F = J/2)
NCORES = 8
D = 18          # Chebyshev degree
K0 = 6          # steps k >= K0 run in fp16, k < K0 in fp32
NNODES = 128
WARP_A = 0.35
SRANGE = 5.5

_CONSTS = None
_NC_CACHE = {}


def _host_constants():
    """Data-independent fit constants (nodes, sign-folded fit matrix)."""
    global _CONSTS
    if _CONSTS is not None:
        return _CONSTS
    import numpy.polynomial.chebyshev as _C

    tmax = float(np.tanh(WARP_A * SRANGE))
    th = (np.arange(NNODES) + 0.5) * np.pi / NNODES
    un = np.cos(th)  # Chebyshev points in [-1,1]
    sn = np.arctanh(un * tmax) / WARP_A  # node s-values
    V = _C.chebvander(un, D)  # (N, D+1)
    G = np.linalg.pinv(V)  # (D+1, N): node values -> cheb coeffs

    # Clenshaw sign schedule: A_k = eps_k * b_k with eps_k = -eps_{k+2} so
    # that each step is one scalar_tensor_tensor: A_k = (A_{k+2} + eps_k*c_k)
    # op1 (2u (*) A_{k+1}).  Require eps_2 = -1 for the final step.
    eps = {}
    for chain in (list(range(2, D + 1, 2))[::-1], list(range(1, D + 1, 2))[::-1]):
        n = len(chain)
        top = (-1) ** n if chain[-1] == 2 else 1
        for i, k in enumerate(chain):
            eps[k] = top * ((-1) ** i)
    sigma = np.array([1] + [eps[k] for k in range(1, D + 1)], dtype=np.float64)
    Gs = G * sigma[:, None]
    gt = np.ascontiguousarray(Gs.T.astype(np.float32))  # (N, D+1)
    _CONSTS = (tmax, sn.astype(np.float32).reshape(NNODES, 1), gt, eps)
    return _CONSTS


def _mock_core(s_tile, qb_tile, sn, gt, tmax, eps):
    """Pure-numpy f32/fp16 mirror of the fast device program (debugging)."""
    f = np.float32
    h = np.float16
    En = np.exp(qb_tile * sn).astype(f)
    S0n = En.sum(1, dtype=f).reshape(-1, 1)
    S1n = (En * qb_tile).sum(1, dtype=f).reshape(-1, 1)
    ctxn = (S1n * (f(1) / S0n)).astype(f)
    gaten = (f(0.5) * np.tanh(f(0.5) * ctxn) + f(0.5)).astype(f)
    c = (gaten[:, 0].astype(f) @ gt).astype(f)  # (D+1,)
    T = np.tanh(f(WARP_A) * s_tile).astype(f)
    U2 = (T * f(2.0 / tmax)).astype(f)
    U1 = (T * f(eps[1] / tmax)).astype(f)
    U2h = U2.astype(h)
    A = {D + 1: np.zeros(s_tile.shape, h), D: np.full(s_tile.shape, c[D], dtype=h)}
    for k in range(D - 1, 0, -1):
        if k >= K0:
            tmp = (U2h.astype(f) * A[k + 1].astype(f)).astype(h)
            x = (A[k + 2].astype(f) + c[k]).astype(h)
            y = x.astype(f) + tmp.astype(f) if eps[k] * eps[k + 1] == 1 \
                else x.astype(f) - tmp.astype(f)
            A[k] = y.astype(h)
        else:
            a1 = A[k + 1].astype(f)
            a2 = A[k + 2].astype(f)
            tmp = (U2 * a1).astype(f)
            x = (a2 + c[k]).astype(f)
            A[k] = (x + tmp).astype(f) if eps[k] * eps[k + 1] == 1 \
                else (x - tmp).astype(f)
    tmpf = (U1 * A[1].astype(f)).astype(f)
    g = ((A[2].astype(f) + c[0]).astype(f) + tmpf).astype(f)
    return (s_tile * g).astype(f)


def _build_nc(variant):
    import concourse.bacc as bacc
    import concourse.mybir as mybir
    from concourse import tile

    fp32 = mybir.dt.float32
    fp16 = mybir.dt.float16
    AF = mybir.ActivationFunctionType
    OP = mybir.AluOpType
    tmax, _sn, _gt, eps = _host_constants()

    nc = bacc.Bacc("TRN2", target_bir_lowering=False, debug=False, num_devices=NCORES)
    s_d = nc.dram_tensor("s", (P, F), fp32, kind="ExternalInput")
    qp_d = nc.dram_tensor("qpair", (2, K), fp16, kind="ExternalInput")
    sn_d = nc.dram_tensor("sn", (NNODES, 1), fp32, kind="ExternalInput")
    gt_d = nc.dram_tensor("gt", (NNODES, D + 1), fp32, kind="ExternalInput")
    y_d = nc.dram_tensor("y", (P, F), fp32, kind="ExternalOutput")

    with tile.TileContext(nc) as tc:
        with (
            tc.tile_pool(name="c1", bufs=1) as cp,
            tc.tile_pool(name="ab", bufs=K0 + 2) as ab,
            tc.tile_pool(name="abh", bufs=D - K0 + 2) as abh,
            tc.tile_pool(name="xh", bufs=D - K0 + 2) as xh,
            tc.tile_pool(name="tm", bufs=K0 + 2) as tp,
            tc.tile_pool(name="tmh", bufs=D - K0 + 2) as tph,
            tc.tile_pool(name="wk", bufs=3) as wp,
            tc.tile_pool(name="ps", bufs=1, space="PSUM") as pp,
        ):
            # hoist the ~1.3us activation-table load into the DMA window: a
            # 1-element activation whose only dep is a local memset makes
            # walrus place PSEUDO_LOAD_ACT_FUNC_SET at t~0
            zz = cp.tile([1, 1], fp32, tag="zz")
            nc.gpsimd.memset(zz[:], 0.0)
            zz2 = cp.tile([1, 1], fp32, tag="zz2")
            nc.scalar.activation(zz2[:], zz[:], AF.Exp)

            # qpair via the Pool SWDGE (25ns seq issue vs 565 on SP) so the
            # fit pipeline starts sooner; SP queue then leads with the bulk s
            qp_sb = cp.tile([2, K], fp16, tag="qp_sb")
            nc.gpsimd.dma_start(out=qp_sb[:], in_=qp_d[:])
            snt = cp.tile([NNODES, 1], fp32, tag="snt")
            nc.sync.dma_start(out=snt[:], in_=sn_d[:])
            s_all = cp.tile([P, F], fp32, tag="s_all")
            nc.sync.dma_start(out=s_all[:], in_=s_d[:])
            gtt = cp.tile([NNODES, D + 1], fp32, tag="gtt")
            nc.sync.dma_start(out=gtt[:], in_=gt_d[:])

            # broadcast q to all 128 partitions exactly: q = qhi + qlo as an
            # fp16 pair summed by a single C=2 matmul into fp32 PSUM
            ones = cp.tile([2, P], fp16, tag="ones")
            nc.gpsimd.memset(ones[:], 1.0)
            q_ps = pp.tile([P, K], fp32, tag="qps")
            nc.tensor.matmul(q_ps[:], ones[:], qp_sb[:], start=True, stop=True)

            if variant == "fast":
                # ---- evaluate g at the fixed nodes (one partition each)
                En = cp.tile([NNODES, K], fp32, tag="En")
                S0n = cp.tile([NNODES, 1], fp32, tag="S0n")
                nc.scalar.activation(
                    En[:], q_ps[:], AF.Exp, scale=snt[:], accum_out=S0n[:]
                )
                # warp for the main tile, emitted early so ACT runs it right
                # after the node exp while the DVE digests the node sums
                T = cp.tile([P, F], fp32, tag="T")
                nc.scalar.activation(T[:], s_all[:], AF.Tanh, scale=float(WARP_A))
                scrn = cp.tile([NNODES, K], fp32, tag="scrn")
                S1n = cp.tile([NNODES, 1], fp32, tag="S1n")
                nc.vector.scalar_tensor_tensor(
                    out=scrn[:], in0=En[:], scalar=1.0, in1=q_ps[:],
                    op0=OP.mult, op1=OP.mult, accum_out=S1n[:],
                )
                recn = cp.tile([NNODES, 1], fp32, tag="recn")
                nc.vector.reciprocal(recn[:], S0n[:])
                ctxn = cp.tile([NNODES, 1], fp32, tag="ctxn")
                nc.vector.tensor_tensor(ctxn[:], S1n[:], recn[:], OP.mult)
                thn = cp.tile([NNODES, 1], fp32, tag="thn")
                nc.scalar.activation(thn[:], ctxn[:], AF.Tanh, scale=0.5)
                gaten = cp.tile([NNODES, 1], fp32, tag="gaten")
                nc.vector.tensor_scalar(
                    out=gaten[:], in0=thn[:], scalar1=0.5, scalar2=0.5,
                    op0=OP.mult, op1=OP.add,
                )
                # broadcast gate along free dim, then PE-contract with gt to
                # land the coefficients replicated on all 128 partitions
                gbf = cp.tile([P, P], fp32, tag="gbf")
                nc.vector.tensor_scalar(
                    out=gbf[:], in0=En[:, 0:P], scalar1=0.0, scalar2=gaten[:],
                    op0=OP.mult, op1=OP.add,
                )
                c_ps = pp.tile([P, D + 1], fp32, tag="cps")
                nc.tensor.matmul(c_ps[:], gbf[:], gtt[:], start=True, stop=True)
                c_sb = cp.tile([P, D + 1], fp32, tag="csb")
                nc.vector.tensor_copy(c_sb[:], c_ps[:])

                # ---- main evaluation over the whole (P,F) tile
                U2 = cp.tile([P, F], fp32, tag="U2")
                nc.vector.tensor_scalar(
                    out=U2[:], in0=T[:], scalar1=float(2.0 / tmax), scalar2=None,
                    op0=OP.mult,
                )
                U2h = cp.tile([P, F], fp16, tag="U2h")
                nc.vector.tensor_copy(U2h[:], U2[:])

                A = {}
                aD = abh.tile([P, F], fp16, tag="Ah")
                nc.vector.tensor_scalar(
                    out=aD[:], in0=En[:], scalar1=0.0, scalar2=c_sb[:, D:D + 1],
                    op0=OP.mult, op1=OP.add,
                )
                A[D] = aD
                h_impl = os.environ.get("BASS_FP16_STEP", "act")
                for k in range(D - 1, 0, -1):
                    op_add = eps[k] * eps[k + 1] == 1
                    if k >= K0:
                        tmp = tph.tile([P, F], fp16, tag="tmph")
                        nc.vector.tensor_tensor(tmp[:], U2h[:], A[k + 1][:], OP.mult)
                        ak = abh.tile([P, F], fp16, tag="Ah")
                        if k == D - 1:
                            # A_{k+2} is identically zero: x = c_k broadcast,
                            # cheap 4x-mode ts on the DVE
                            x = xh.tile([P, F], fp16, tag="X")
                            nc.vector.tensor_scalar(
                                out=x[:], in0=En[:], scalar1=0.0,
                                scalar2=c_sb[:, k:k + 1], op0=OP.mult, op1=OP.add,
                            )
                            nc.vector.tensor_tensor(
                                ak[:], x[:], tmp[:],
                                OP.add if op_add else OP.subtract,
                            )
                        elif h_impl == "act":
                            # the "+c_k" runs on the otherwise idle ACT engine
                            # (2 steps of slack), DVE does two 2x-mode tts
                            x = xh.tile([P, F], fp16, tag="X")
                            nc.scalar.activation(
                                x[:], A[k + 2][:], AF.Identity,
                                bias=c_sb[:, k:k + 1], scale=1.0,
                            )
                            nc.vector.tensor_tensor(
                                ak[:], x[:], tmp[:],
                                OP.add if op_add else OP.subtract,
                            )
                        elif h_impl == "ts":
                            x = xh.tile([P, F], fp16, tag="X")
                            nc.vector.tensor_scalar(
                                out=x[:], in0=A[k + 2][:],
                                scalar1=c_sb[:, k:k + 1], scalar2=None,
                                op0=OP.add,
                            )
                            nc.vector.tensor_tensor(
                                ak[:], x[:], tmp[:],
                                OP.add if op_add else OP.subtract,
                            )
                        else:  # stt
                            nc.vector.scalar_tensor_tensor(
                                out=ak[:], in0=A[k + 2][:],
                                scalar=c_sb[:, k:k + 1], in1=tmp[:],
                                op0=OP.add,
                                op1=OP.add if op_add else OP.subtract,
                            )
                    else:
                        tmp = tp.tile([P, F], fp32, tag="tmp")
                        nc.vector.tensor_tensor(tmp[:], U2[:], A[k + 1][:], OP.mult)
                        ak = ab.tile([P, F], fp32, tag="A")
                        nc.vector.scalar_tensor_tensor(
                            out=ak[:], in0=A[k + 2][:], scalar=c_sb[:, k:k + 1],
                            in1=tmp[:], op0=OP.add,
                            op1=OP.add if op_add else OP.subtract,
                        )
                    A[k] = ak
                # final: p = c_0 + u*b_1 - b_2, split into column halves so the
                # first half's store overlaps the second half's compute
                U1 = cp.tile([P, F], fp32, tag="U1")
                nc.vector.tensor_scalar(
                    out=U1[:], in0=T[:], scalar1=float(eps[1] / tmax), scalar2=None,
                    op0=OP.mult,
                )
                outt = cp.tile([P, F], fp32, tag="outt")
                H = F // 2
                for hcol in range(2):
                    sl = slice(hcol * H, (hcol + 1) * H)
                    tmpf = tp.tile([P, H], fp32, tag="tmp")
                    nc.vector.tensor_tensor(tmpf[:], U1[:, sl], A[1][:, sl], OP.mult)
                    g = wp.tile([P, H], fp32, tag="g")
                    nc.vector.scalar_tensor_tensor(
                        out=g[:], in0=A[2][:, sl], scalar=c_sb[:, 0:1],
                        in1=tmpf[:], op0=OP.add, op1=OP.add,
                    )
                    nc.vector.tensor_tensor(outt[:, sl], s_all[:, sl], g[:], OP.mult)
                    nc.sync.dma_start(out=y_d[:, sl], in_=outt[:, sl])
            else:
                S0 = cp.tile([P, F], fp32, tag="S0")
                S1 = cp.tile([P, F], fp32, tag="S1")
                qb = cp.tile([P, K], fp32, tag="qb")
                nc.vector.tensor_copy(qb[:], q_ps[:])
                for j in range(F):
                    E = wp.tile([P, K], fp32, tag="E")
                    nc.scalar.activation(
                        E[:], qb[:], AF.Exp, scale=s_all[:, j:j + 1],
                        accum_out=S0[:, j:j + 1],
                    )
                    scr = wp.tile([P, K], fp32, tag="scr")
                    nc.vector.scalar_tensor_tensor(
                        out=scr[:], in0=E[:], scalar=1.0, in1=qb[:],
                        op0=OP.mult, op1=OP.mult, accum_out=S1[:, j:j + 1],
                    )
                rec = cp.tile([P, F], fp32, tag="rec")
                nc.vector.reciprocal(rec[:], S0[:])
                ctx = cp.tile([P, F], fp32, tag="ctx")
                nc.vector.tensor_tensor(ctx[:], S1[:], rec[:], OP.mult)
                th = cp.tile([P, F], fp32, tag="th")
                nc.scalar.activation(th[:], ctx[:], AF.Tanh, scale=0.5)
                gate = cp.tile([P, F], fp32, tag="gate")
                nc.vector.tensor_scalar(
                    out=gate[:], in0=th[:], scalar1=0.5, scalar2=0.5,
                    op0=OP.mult, op1=OP.add,
                )
                outt = cp.tile([P, F], fp32, tag="outt")
                nc.vector.tensor_tensor(outt[:], s_all[:], gate[:], OP.mult)
                nc.sync.dma_start(out=y_d[:], in_=outt[:])

    nc.compile()
    return nc


def _get_nc(variant):
    if variant not in _NC_CACHE:
        _NC_CACHE[variant] = _build_nc(variant)
    return _NC_CACHE[variant]


def _in_maps(key, query):
    _tmax, sn, gt, _eps = _host_constants()
    s2 = key.reshape(B, J)
    h = J // 2
    maps = []
    for c in range(NCORES):
        b, half = divmod(c, 2)
        q = query[b].astype(np.float32)
        qhi = q.astype(np.float16)
        qlo = (q - qhi.astype(np.float32)).astype(np.float16)
        maps.append({
            "s": np.ascontiguousarray(s2[b, half * h:(half + 1) * h].reshape(P, F)),
            "qpair": np.ascontiguousarray(np.stack([qhi, qlo], 0)),
            "sn": sn,
            "gt": gt,
        })
    return maps


def kernel(key, query, _variant=None, _trace=False):
    key = np.ascontiguousarray(key, dtype=np.float32)
    query = np.ascontiguousarray(query, dtype=np.float32)
    variant = _variant or os.environ.get("BASS_KERNEL_VARIANT", "fast")
    nc = _get_nc(variant)
    from concourse.bass_utils import run_bass_kernel_spmd

    res = run_bass_kernel_spmd(
        nc, _in_maps(key, query), list(range(NCORES)), trace=_trace
    )
    h = J // 2
    out = np.empty((B, J), np.float32)
    for c in range(NCORES):
        b, half = divmod(c, 2)
        out[b, half * h:(half + 1) * h] = res.results[c]["y"].reshape(h)
    if _trace:
        kernel.last_results = res
    return out.reshape(key.shape)



# revision 6
# speedup vs baseline: 2.3493x; 2.3493x over previous
"""Trainium2 Bass kernel for nn_AttentionMask_13048110645633.

Math: for key (4,32,64,64) and query (4,512), with s = key.reshape(B,J) and
q = query, the reference computes elementwise

    ctx[b,j] = sum_k q[b,k]*exp(s[b,j]*q[b,k]) / sum_k exp(s[b,j]*q[b,k])
    out[b,j] = s[b,j] * sigmoid(ctx[b,j])

i.e. out = s * g_b(s) with g_b a smooth scalar gate determined by q[b].
Sharding: data-parallel over B (4 batches x 2 half-slabs = 8 cores), one
(128,512) tile per core.

Device algorithm (per core):
  fit phase  - 128 fit nodes s_n synthesized on-device (iota+affine, no DMA);
               q broadcast to all partitions by a C=2 fp16-pair PE matmul;
               one ACT exp with per-partition scale + accumulate gives
               En=exp(s_n q) and S0; one DVE stt multiply-reduce gives
               0.5*S1; tanh(S1/(2*S0)) on ACT; the sigmoid affine and the
               least-squares fit both fold into a host fit matrix applied by
               two tiny PE matmuls -> even/odd polynomial coefficients of
               g in u = tanh(0.4 s), replicated on all 128 partitions.
  eval phase - u from one ACT tanh (fp16); v=u^2, v2=v^4 by DVE tt; the
               degree-7 polynomial evaluated Estrin-style with 4x-mode fp16
               tensor_scalar ops (per-partition coefficient pairs) and
               2x-mode fp16 tensor_tensor ops; final out = s*g in two fp32
               column halves.
  stores     - kv_writeback descriptors prepared on the SWDGE ring during
               the fit phase; each output half fires via trigger_dma as soon
               as its final multiply lands (no HWDGE issue latency on the
               tail).
"""

import os
import numpy as np

B, J, K = 4, 131072, 512
P, F = 128, 512   # per-core tile (P*F = J/2)
H = F // 2        # output store half-columns
NCORES = 8
NN = 128          # fit nodes (one per partition)
D = 7             # polynomial degree in u
WA = 0.4          # tanh warp: u = tanh(WA*s)
SR = 5.2          # node range: s_n uniform in (-SR, SR)
NE = D // 2 + 1   # even-part coeffs (poly in v=u^2)
NO = (D + 1) // 2 # odd-part coeffs
NC_ = NE + NO

_CONSTS = None
_NC_CACHE = {}


def _host_constants():
    """Data-independent fit constants: node affine map + folded fit matrix."""
    global _CONSTS
    if _CONSTS is not None:
        return _CONSTS
    alpha = 2.0 * SR / NN
    beta = -SR + SR / NN           # s_n = beta + alpha*n,  n = 0..NN-1
    n = np.arange(NN, dtype=np.float64)
    un = np.tanh(WA * (beta + alpha * n))
    vn = un * un
    Vb = np.concatenate([
        np.stack([vn**m for m in range(NE)], 1),
        np.stack([un * vn**m for m in range(NO)], 1)], 1)   # (NN, NC_)
    G = np.linalg.pinv(Vb)                                   # (NC_, NN)
    # gate = 0.5*tanh(0.5*ctx) + 0.5; the affine folds into the fit:
    # c = G @ gate = (0.5*G) @ tanhvals + G @ (0.5*ones)
    gt2 = np.ascontiguousarray((0.5 * G).T.astype(np.float16))   # (NN, NC_)
    coff = np.ascontiguousarray(
        (G @ (0.5 * np.ones(NN))).astype(np.float16).reshape(1, NC_))
    _CONSTS = (float(alpha), float(beta), gt2, coff)
    return _CONSTS


def _build_nc(variant):
    import concourse.bacc as bacc
    import concourse.bass as bass_mod
    import concourse.mybir as mybir
    from concourse import tile

    fp32 = mybir.dt.float32
    fp16 = mybir.dt.float16
    i32 = mybir.dt.int32
    AF = mybir.ActivationFunctionType
    OP = mybir.AluOpType
    alpha, beta, _gt2, _coff = _host_constants()

    nc = bacc.Bacc("TRN2", target_bir_lowering=False, debug=False,
                   num_devices=NCORES, num_swdge_queues=3)
    s_d = nc.dram_tensor("s", (P, F), fp32, kind="ExternalInput")
    qp_d = nc.dram_tensor("qpair", (2, K), fp16, kind="ExternalInput")
    gt_d = nc.dram_tensor("gt2", (NN, NC_), fp16, kind="ExternalInput")
    co_d = nc.dram_tensor("coff", (1, NC_), fp16, kind="ExternalInput")
    y_d = nc.dram_tensor("y", (P, F), fp32, kind="ExternalOutput")

    sem0 = nc.alloc_semaphore("wb0")
    sem1 = nc.alloc_semaphore("wb1")

    with tile.TileContext(nc) as tc:
        with (
            tc.tile_pool(name="c1", bufs=1) as cp,
            tc.tile_pool(name="ps", bufs=2, space="PSUM") as pp,
        ):
            # ---------------- Pool (gpsimd) queue: loads + metadata --------
            qp_sb = cp.tile([2, K], fp16, tag="qp_sb")
            nc.gpsimd.dma_start(out=qp_sb[:], in_=qp_d[:])
            zz = cp.tile([1, 1], fp32, tag="zz")
            nc.gpsimd.memset(zz[:], 0.0)
            io = cp.tile([P, 1], fp32, tag="io")
            nc.gpsimd.iota(io[:], [[1, 1]], channel_multiplier=1,
                           allow_small_or_imprecise_dtypes=True)
            ones2 = cp.tile([2, P], fp16, tag="ones2")
            nc.gpsimd.memset(ones2[:], 1.0)
            ones1 = cp.tile([1, P], fp16, tag="ones1")
            nc.gpsimd.memset(ones1[:], 1.0)
            cidx = cp.tile([P, 1], i32, tag="cidx")
            nc.gpsimd.memset(cidx[:], 0)

            # ---------------- SP queue: bulk + fit-matrix loads ------------
            s_all = cp.tile([P, F], fp32, tag="s_all")
            nc.sync.dma_start(out=s_all[:], in_=s_d[:])
            gtt = cp.tile([NN, NC_], fp16, tag="gtt")
            nc.sync.dma_start(out=gtt[:], in_=gt_d[:])
            coft = cp.tile([1, NC_], fp16, tag="coft")
            nc.sync.dma_start(out=coft[:], in_=co_d[:])

            # output tile + store descriptor prep (SWDGE ring, data deferred)
            outt = cp.tile([P, F], fp32, tag="outt")

            def wb(prep_q, sem, col0):
                oh = outt[:, col0:col0 + H]
                in4 = bass_mod.AP(oh.tensor, oh.offset,
                                  [list(oh.ap[0]), [F, 1], [F, 1],
                                   list(oh.ap[-1])])
                ya = y_d[:]
                out4 = bass_mod.AP(ya.tensor, ya.offset + col0,
                                   [[P * F, 1], [F, P], [F, 1], [1, H]])
                return nc.gpsimd.kv_writeback(
                    out4, in4, cidx[:],
                    prepare_only=True, sem=sem, queue_num=prep_q)

            # ---------------- ACT warmup: hoist the act-table load --------
            zz2 = cp.tile([1, 1], fp32, tag="zz2")
            nc.scalar.activation(zz2[:], zz[:], AF.Exp)

            # ---------------- fit-node pipeline ---------------------------
            snt = cp.tile([P, 1], fp32, tag="snt")
            nc.vector.tensor_scalar(out=snt[:], in0=io[:], scalar1=alpha,
                                    scalar2=beta, op0=OP.mult, op1=OP.add)

            q_ps = pp.tile([P, K], fp32, tag="qps")
            nc.tensor.matmul(q_ps[:], ones2[:], qp_sb[:], start=True, stop=True)

            En = cp.tile([NN, K], fp32, tag="En")
            S0n = cp.tile([NN, 1], fp32, tag="S0n")
            nc.scalar.activation(En[:], q_ps[:], AF.Exp, scale=snt[:],
                                 accum_out=S0n[:])
            # warp for the main tile (ACT, right after the node exp)
            T = cp.tile([P, F], fp16, tag="T")
            nc.scalar.activation(T[:], s_all[:], AF.Tanh, scale=float(WA))

            recn = cp.tile([NN, 1], fp32, tag="recn")
            nc.vector.reciprocal(recn[:], S0n[:])
            scr = cp.tile([NN, K], fp32, tag="scr")
            S1n = cp.tile([NN, 1], fp32, tag="S1n")
            nc.vector.scalar_tensor_tensor(
                out=scr[:], in0=En[:], scalar=0.5, in1=q_ps[:],
                op0=OP.mult, op1=OP.mult, accum_out=S1n[:])
            # thn = tanh(0.5*S1/S0) with the 0.5 folded into S1
            thn = cp.tile([NN, 1], fp32, tag="thn")
            nc.scalar.activation(thn[:], S1n[:], AF.Tanh, scale=recn[:])
            # broadcast along free dim for the PE contraction (fp16 lhsT)
            gbf = cp.tile([P, P], fp16, tag="gbf")
            nc.scalar.activation(gbf[:], En[:, 0:P], AF.Identity,
                                 bias=thn[:], scale=0.0)
            c_ps = pp.tile([P, NC_], fp32, tag="cps")
            nc.tensor.matmul(c_ps[:], gbf[:], gtt[:], start=True, stop=False)
            nc.tensor.matmul(c_ps[:], ones1[:], coft[:], start=False, stop=True)

            # ---------------- main evaluation (DVE, fp16) -----------------
            v = cp.tile([P, F], fp16, tag="v")
            nc.vector.tensor_tensor(v[:], T[:], T[:], OP.mult)
            v2 = cp.tile([P, F], fp16, tag="v2")
            nc.vector.tensor_tensor(v2[:], v[:], v[:], OP.mult)

            c_sb = cp.tile([P, NC_], fp32, tag="csb")
            nc.vector.tensor_copy(c_sb[:], c_ps[:])

            def col(i):
                return c_sb[:, i:i + 1]

            a = cp.tile([P, F], fp16, tag="a")
            nc.vector.tensor_scalar(out=a[:], in0=v[:], scalar1=col(1),
                                    scalar2=col(0), op0=OP.mult, op1=OP.add)
            bb = cp.tile([P, F], fp16, tag="bb")
            nc.vector.tensor_scalar(out=bb[:], in0=v[:], scalar1=col(3),
                                    scalar2=col(2), op0=OP.mult, op1=OP.add)
            t1 = cp.tile([P, F], fp16, tag="t1")
            nc.vector.tensor_tensor(t1[:], v2[:], bb[:], OP.mult)
            E = cp.tile([P, F], fp16, tag="E")
            nc.vector.tensor_tensor(E[:], t1[:], a[:], OP.add)
            cc = cp.tile([P, F], fp16, tag="cc")
            nc.vector.tensor_scalar(out=cc[:], in0=v[:], scalar1=col(NE + 1),
                                    scalar2=col(NE), op0=OP.mult, op1=OP.add)
            dd = cp.tile([P, F], fp16, tag="dd")
            nc.vector.tensor_scalar(out=dd[:], in0=v[:], scalar1=col(NE + 3),
                                    scalar2=col(NE + 2), op0=OP.mult, op1=OP.add)
            t3 = cp.tile([P, F], fp16, tag="t3")
            nc.vector.tensor_tensor(t3[:], v2[:], dd[:], OP.mult)
            O = cp.tile([P, F], fp16, tag="O")
            nc.vector.tensor_tensor(O[:], t3[:], cc[:], OP.add)
            t4 = cp.tile([P, F], fp16, tag="t4")
            nc.vector.tensor_tensor(t4[:], T[:], O[:], OP.mult)
            g = cp.tile([P, F], fp16, tag="g")
            nc.vector.tensor_tensor(g[:], E[:], t4[:], OP.add)

            # out = s*g in fp32 column halves; each half fires its store via
            # a pre-prepared SWDGE descriptor (the prep's data read defers to
            # the trigger, so the prep itself runs early on the idle ring)
            nc.vector.tensor_tensor(outt[:, 0:H], s_all[:, 0:H], g[:, 0:H],
                                    OP.mult)
            wb(1, sem0, 0)
            nc.gpsimd.trigger_dma(count=None, queue_num=1)
            nc.vector.tensor_tensor(outt[:, H:F], s_all[:, H:F], g[:, H:F],
                                    OP.mult)
            wb(2, sem1, H)
            nc.gpsimd.trigger_dma(count=None, queue_num=2)
            nc.gpsimd.wait_ge(sem0, 16)
            nc.gpsimd.wait_ge(sem1, 16)

    _patch_store_sync(nc, mybir)
    nc.compile()
    return nc


def _patch_store_sync(nc, mybir):
    """Post-schedule sync fixups for the triggered SWDGE stores.

    Tile places each writeback prep's (deferred) data wait BEFORE the prep,
    serializing the ~1us descriptor generation behind the final compute.  The
    prep only writes ring descriptors — its source read happens at trigger
    time — so the data wait belongs on the trigger.  Move it there.

    Tile's end-of-kernel drain also waits on the DMASW lane sems it assigned
    to the preps, but a prepare_only descriptor carries the caller's sem
    (wb0/wb1, which we wait on explicitly), so those lane sems never move.
    Strip waits on semaphores that no instruction updates.
    """
    fn = nc.m.functions[0]
    insts = [i for b in fn.blocks for i in b.instructions]
    pool = [i for i in insts if i.engine == mybir.EngineType.Pool]

    def waits(i):
        return list(i.sync_info.on_wait) if i.sync_info else []

    def ups(i):
        return list(i.sync_info.on_update) if i.sync_info else []

    def set_sync(i, w, u):
        i.sync_info = mybir.SyncInfo(on_wait=w, on_update=u)

    # move compute->prep waits onto the matching trigger
    for idx, ins in enumerate(pool):
        if type(ins).__name__ != "InstKVWritebackAnt":
            continue
        trig = next(t for t in pool[idx + 1:]
                    if type(t).__name__ == "InstTriggerDma")
        moved = []
        for src in (pool[idx - 1], ins):
            if src is ins or type(src).__name__ == "InstEventSemaphore":
                keep = []
                for w in waits(src):
                    (moved if (w.ant_name or "").startswith("DVE")
                     else keep).append(w)
                if moved or keep != waits(src):
                    set_sync(src, keep, ups(src))
        if moved:
            set_sync(trig, waits(trig) + moved, ups(trig))

    # strip waits on semaphores nothing updates (dead lane sems)
    updated = {u.id for i in insts for u in ups(i)}
    for ins in insts:
        w = waits(ins)
        keep = [x for x in w if x.id in updated]
        if len(keep) != len(w):
            set_sync(ins, keep, ups(ins))


def _get_nc(variant="fast"):
    if variant not in _NC_CACHE:
        _NC_CACHE[variant] = _build_nc(variant)
    return _NC_CACHE[variant]


def _in_maps(key, query):
    _alpha, _beta, gt2, coff = _host_constants()
    s2 = key.reshape(B, J)
    h = J // 2
    maps = []
    for c in range(NCORES):
        b, half = divmod(c, 2)
        q = query[b].astype(np.float32)
        qhi = q.astype(np.float16)
        qlo = (q - qhi.astype(np.float32)).astype(np.float16)
        maps.append({
            "s": np.ascontiguousarray(
                s2[b, half * h:(half + 1) * h].reshape(P, F)),
            "qpair": np.ascontiguousarray(np.stack([qhi, qlo], 0)),
            "gt2": gt2,
            "coff": coff,
        })
    return maps


def kernel(key, query, _variant=None, _trace=False):
    key = np.ascontiguousarray(key, dtype=np.float32)
    query = np.ascontiguousarray(query, dtype=np.float32)
    nc = _get_nc(_variant or "fast")
    from concourse.bass_utils import run_bass_kernel_spmd

    res = run_bass_kernel_spmd(
        nc, _in_maps(key, query), list(range(NCORES)), trace=_trace)
    h = J // 2
    out = np.empty((B, J), np.float32)
    for c in range(NCORES):
        b, half = divmod(c, 2)
        out[b, half * h:(half + 1) * h] = res.results[c]["y"].reshape(h)
    if _trace:
        kernel.last_results = res
    return out.reshape(key.shape)


# revision 10
# speedup vs baseline: 2.4257x; 1.0325x over previous
"""Trainium2 Bass kernel for nn_AttentionMask_13048110645633.

Math: for key (4,32,64,64) and query (4,512), with s = key.reshape(B,J) and
q = query, the reference computes elementwise

    ctx[b,j] = sum_k q[b,k]*exp(s[b,j]*q[b,k]) / sum_k exp(s[b,j]*q[b,k])
    out[b,j] = s[b,j] * sigmoid(ctx[b,j])

i.e. out = s * g_b(s) with g_b a smooth scalar gate determined by q[b].
Sharding: data-parallel over B (4 batches x 2 half-slabs = 8 cores), one
(128,512) tile per core.

Device algorithm (per core):
  fit phase  - 128 fit nodes s_n synthesized on-device (iota+affine, no DMA);
               q broadcast to all partitions by a C=2 fp16-pair PE matmul;
               one ACT exp with per-partition scale + accumulate gives
               En=exp(s_n q) and S0; one DVE stt multiply-reduce gives
               0.5*S1; tanh(S1/(2*S0)) on ACT; the sigmoid affine and the
               least-squares fit both fold into a host fit matrix applied by
               two tiny PE matmuls -> even/odd polynomial coefficients of
               g in u = tanh(0.4 s), replicated on all 128 partitions.
  eval phase - u from one ACT tanh (fp16); v=u^2, v2=v^4 by DVE tt; the
               degree-7 polynomial evaluated Estrin-style with 4x-mode fp16
               tensor_scalar ops (per-partition coefficient pairs) and
               2x-mode fp16 tensor_tensor ops; final out = s*g in two fp32
               column halves.
  stores     - kv_writeback descriptors prepared on the SWDGE ring during
               the fit phase; each output half fires via trigger_dma as soon
               as its final multiply lands (no HWDGE issue latency on the
               tail).
"""

import os
import numpy as np

B, J, K = 4, 131072, 512
P, F = 128, 512   # per-core tile (P*F = J/2)
H = F // 2        # output store half-columns
NCORES = 8
NN = 128          # fit nodes (one per partition)
D = 7             # polynomial degree in u
WA = 0.4          # tanh warp: u = tanh(WA*s)
SR = 5.2          # node range: s_n uniform in (-SR, SR)
NE = D // 2 + 1   # even-part coeffs (poly in v=u^2)
NO = (D + 1) // 2 # odd-part coeffs
NC_ = NE + NO

_CONSTS = None
_NC_CACHE = {}


def _host_constants():
    """Data-independent fit constants: node affine map + folded fit matrix."""
    global _CONSTS
    if _CONSTS is not None:
        return _CONSTS
    alpha = 2.0 * SR / NN
    beta = -SR + SR / NN           # s_n = beta + alpha*n,  n = 0..NN-1
    n = np.arange(NN, dtype=np.float64)
    un = np.tanh(WA * (beta + alpha * n))
    vn = un * un
    Vb = np.concatenate([
        np.stack([vn**m for m in range(NE)], 1),
        np.stack([un * vn**m for m in range(NO)], 1)], 1)   # (NN, NC_)
    G = np.linalg.pinv(Vb)                                   # (NC_, NN)
    # gate = 0.5*tanh(0.5*ctx) + 0.5; the affine folds into the fit:
    # c = G @ gate = (0.5*G) @ tanhvals + G @ (0.5*ones)
    gt2 = np.ascontiguousarray((0.5 * G).T.astype(np.float16))   # (NN, NC_)
    coff = np.ascontiguousarray(
        (G @ (0.5 * np.ones(NN))).astype(np.float16).reshape(1, NC_))
    _CONSTS = (float(alpha), float(beta), gt2, coff)
    return _CONSTS


def _build_nc(variant):
    import concourse.bacc as bacc
    import concourse.bass as bass_mod
    import concourse.mybir as mybir
    from concourse import tile

    fp32 = mybir.dt.float32
    fp16 = mybir.dt.float16
    i32 = mybir.dt.int32
    AF = mybir.ActivationFunctionType
    OP = mybir.AluOpType
    alpha, beta, _gt2, _coff = _host_constants()

    nc = bacc.Bacc("TRN2", target_bir_lowering=False, debug=False,
                   num_devices=NCORES, num_swdge_queues=3)
    s_d = nc.dram_tensor("s", (P, F), fp32, kind="ExternalInput")
    qp_d = nc.dram_tensor("qpair", (2, K), fp16, kind="ExternalInput")
    gt_d = nc.dram_tensor("gt2", (NN, NC_), fp16, kind="ExternalInput")
    co_d = nc.dram_tensor("coff", (1, NC_), fp16, kind="ExternalInput")
    y_d = nc.dram_tensor("y", (P, F), fp32, kind="ExternalOutput")

    sem0 = nc.alloc_semaphore("wb0")
    sem1 = nc.alloc_semaphore("wb1")

    with tile.TileContext(nc) as tc:
        with (
            tc.tile_pool(name="c1", bufs=1) as cp,
            tc.tile_pool(name="ps", bufs=2, space="PSUM") as pp,
        ):
            # ---------------- Pool (gpsimd) queue: loads + metadata --------
            # bulk s tile via SWDGE (Pool prologue delay hides under the
            # longer q->En critical path); q goes first on SP-HWDGE
            s_all = cp.tile([P, F], fp32, tag="s_all")
            nc.gpsimd.dma_start(out=s_all[:], in_=s_d[:])
            zz = cp.tile([1, 1], fp32, tag="zz")
            nc.gpsimd.memset(zz[:], 0.0)
            io = cp.tile([P, 1], fp32, tag="io")
            nc.gpsimd.iota(io[:], [[1, 1]], channel_multiplier=1,
                           allow_small_or_imprecise_dtypes=True)
            ones2 = cp.tile([2, P], fp16, tag="ones2")
            nc.gpsimd.memset(ones2[:], 1.0)
            ones1 = cp.tile([1, P], fp16, tag="ones1")
            nc.gpsimd.memset(ones1[:], 1.0)
            cidx = cp.tile([P, 1], i32, tag="cidx")
            nc.gpsimd.memset(cidx[:], 0)

            # ---------------- SP queue: q + fit-matrix loads ---------------
            qp_sb = cp.tile([2, K], fp16, tag="qp_sb")
            nc.sync.dma_start(out=qp_sb[:], in_=qp_d[:])
            gtt = cp.tile([NN, NC_], fp16, tag="gtt")
            nc.sync.dma_start(out=gtt[:], in_=gt_d[:])
            coft = cp.tile([1, NC_], fp16, tag="coft")
            nc.sync.dma_start(out=coft[:], in_=co_d[:])

            # output tile + store descriptor prep (SWDGE ring, data deferred)
            outt = cp.tile([P, F], fp32, tag="outt")

            def wb(prep_q, sem, col0):
                oh = outt[:, col0:col0 + H]
                in4 = bass_mod.AP(oh.tensor, oh.offset,
                                  [list(oh.ap[0]), [F, 1], [F, 1],
                                   list(oh.ap[-1])])
                ya = y_d[:]
                out4 = bass_mod.AP(ya.tensor, ya.offset + col0,
                                   [[P * F, 1], [F, P], [F, 1], [1, H]])
                return nc.gpsimd.kv_writeback(
                    out4, in4, cidx[:],
                    prepare_only=True, sem=sem, queue_num=prep_q)

            # ---------------- ACT warmup: hoist the act-table load --------
            zz2 = cp.tile([1, 1], fp32, tag="zz2")
            nc.scalar.activation(zz2[:], zz[:], AF.Exp)

            # ---------------- fit-node pipeline ---------------------------
            snt = cp.tile([P, 1], fp32, tag="snt")
            nc.vector.tensor_scalar(out=snt[:], in0=io[:], scalar1=alpha,
                                    scalar2=beta, op0=OP.mult, op1=OP.add)

            q_ps = pp.tile([P, K], fp32, tag="qps")
            nc.tensor.matmul(q_ps[:], ones2[:], qp_sb[:], start=True, stop=True)

            En = cp.tile([NN, K], fp32, tag="En")
            S0n = cp.tile([NN, 1], fp32, tag="S0n")
            nc.scalar.activation(En[:], q_ps[:], AF.Exp, scale=snt[:],
                                 accum_out=S0n[:])
            # warp for the main tile (ACT, right after the node exp)
            T = cp.tile([P, F], fp16, tag="T")
            nc.scalar.activation(T[:], s_all[:], AF.Tanh, scale=float(WA))

            recn = cp.tile([NN, 1], fp32, tag="recn")
            nc.vector.reciprocal(recn[:], S0n[:])
            scr = cp.tile([NN, K], fp32, tag="scr")
            S1n = cp.tile([NN, 1], fp32, tag="S1n")
            nc.vector.scalar_tensor_tensor(
                out=scr[:], in0=En[:], scalar=0.5, in1=q_ps[:],
                op0=OP.mult, op1=OP.mult, accum_out=S1n[:])
            # thn = tanh(0.5*S1/S0) with the 0.5 folded into S1
            thn = cp.tile([NN, 1], fp32, tag="thn")
            nc.scalar.activation(thn[:], S1n[:], AF.Tanh, scale=recn[:])
            # broadcast along free dim for the PE contraction (fp16 lhsT)
            gbf = cp.tile([P, P], fp16, tag="gbf")
            nc.scalar.activation(gbf[:], En[:, 0:P], AF.Identity,
                                 bias=thn[:], scale=0.0)
            c_ps = pp.tile([P, NC_], fp32, tag="cps")
            nc.tensor.matmul(c_ps[:], gbf[:], gtt[:], start=True, stop=False)
            nc.tensor.matmul(c_ps[:], ones1[:], coft[:], start=False, stop=True)

            # ---------------- main evaluation (DVE, fp16) -----------------
            v = cp.tile([P, F], fp16, tag="v")
            nc.vector.tensor_tensor(v[:], T[:], T[:], OP.mult)
            v2 = cp.tile([P, F], fp16, tag="v2")
            nc.vector.tensor_tensor(v2[:], v[:], v[:], OP.mult)

            c_sb = cp.tile([P, NC_], fp32, tag="csb")
            nc.vector.tensor_copy(c_sb[:], c_ps[:])

            def col(i):
                return c_sb[:, i:i + 1]

            a = cp.tile([P, F], fp16, tag="a")
            nc.vector.tensor_scalar(out=a[:], in0=v[:], scalar1=col(1),
                                    scalar2=col(0), op0=OP.mult, op1=OP.add)
            bb = cp.tile([P, F], fp16, tag="bb")
            nc.vector.tensor_scalar(out=bb[:], in0=v[:], scalar1=col(3),
                                    scalar2=col(2), op0=OP.mult, op1=OP.add)
            t1 = cp.tile([P, F], fp16, tag="t1")
            nc.vector.tensor_tensor(t1[:], v2[:], bb[:], OP.mult)
            E = cp.tile([P, F], fp16, tag="E")
            nc.vector.tensor_tensor(E[:], t1[:], a[:], OP.add)
            cc = cp.tile([P, F], fp16, tag="cc")
            nc.vector.tensor_scalar(out=cc[:], in0=v[:], scalar1=col(NE + 1),
                                    scalar2=col(NE), op0=OP.mult, op1=OP.add)
            dd = cp.tile([P, F], fp16, tag="dd")
            nc.vector.tensor_scalar(out=dd[:], in0=v[:], scalar1=col(NE + 3),
                                    scalar2=col(NE + 2), op0=OP.mult, op1=OP.add)
            t3 = cp.tile([P, F], fp16, tag="t3")
            nc.vector.tensor_tensor(t3[:], v2[:], dd[:], OP.mult)
            O = cp.tile([P, F], fp16, tag="O")
            nc.vector.tensor_tensor(O[:], t3[:], cc[:], OP.add)
            t4 = cp.tile([P, F], fp16, tag="t4")
            nc.vector.tensor_tensor(t4[:], T[:], O[:], OP.mult)
            g = cp.tile([P, F], fp16, tag="g")
            nc.vector.tensor_tensor(g[:], E[:], t4[:], OP.add)

            # out = s*g in fp32 column halves; each half fires its store via
            # a pre-prepared SWDGE descriptor (the prep's data read defers to
            # the trigger, so the preps themselves run early on the idle ring
            # once _patch_store_sync moves their data waits onto the triggers)
            nc.vector.tensor_tensor(outt[:, 0:H], s_all[:, 0:H], g[:, 0:H],
                                    OP.mult)
            nc.vector.tensor_tensor(outt[:, H:F], s_all[:, H:F], g[:, H:F],
                                    OP.mult)
            wb(1, sem0, 0)
            wb(2, sem1, H)
            nc.gpsimd.trigger_dma(count=None, queue_num=1)
            nc.gpsimd.trigger_dma(count=None, queue_num=2)
            nc.gpsimd.wait_ge(sem0, 16)
            nc.gpsimd.wait_ge(sem1, 16)

    _patch_store_sync(nc, mybir)
    nc.compile()
    return nc


def _patch_store_sync(nc, mybir):
    """Post-schedule sync fixups for the triggered SWDGE stores.

    Tile places each writeback prep's (deferred) data wait BEFORE the prep,
    serializing the ~1us descriptor generation behind the final compute.  The
    prep only writes ring descriptors — its source read happens at trigger
    time — so the data wait belongs on the trigger.  Move it there.

    Tile's end-of-kernel drain also waits on the DMASW lane sems it assigned
    to the preps, but a prepare_only descriptor carries the caller's sem
    (wb0/wb1, which we wait on explicitly), so those lane sems never move.
    Strip waits on semaphores that no instruction updates.
    """
    fn = nc.m.functions[0]
    insts = [i for b in fn.blocks for i in b.instructions]
    pool = [i for i in insts if i.engine == mybir.EngineType.Pool]

    def waits(i):
        return list(i.sync_info.on_wait) if i.sync_info else []

    def ups(i):
        return list(i.sync_info.on_update) if i.sync_info else []

    def set_sync(i, w, u):
        i.sync_info = mybir.SyncInfo(on_wait=w, on_update=u)

    # move compute->prep waits onto the same-queue trigger
    for idx, ins in enumerate(pool):
        if type(ins).__name__ != "InstKVWritebackAnt":
            continue
        trig = next(t for t in pool[idx + 1:]
                    if type(t).__name__ == "InstTriggerDma"
                    and t.queue_num == ins.queue_num)
        moved = []
        for src in (pool[idx - 1], ins):
            if src is ins or type(src).__name__ == "InstEventSemaphore":
                keep = []
                for w in waits(src):
                    (moved if (w.ant_name or "").startswith("DVE")
                     else keep).append(w)
                if moved or keep != waits(src):
                    set_sync(src, keep, ups(src))
        if moved:
            set_sync(trig, waits(trig) + moved, ups(trig))

    # strip waits on semaphores nothing updates (dead lane sems)
    updated = {u.id for i in insts for u in ups(i)}
    for ins in insts:
        w = waits(ins)
        keep = [x for x in w if x.id in updated]
        if len(keep) != len(w):
            set_sync(ins, keep, ups(ins))


def _get_nc(variant="fast"):
    if variant not in _NC_CACHE:
        _NC_CACHE[variant] = _build_nc(variant)
    return _NC_CACHE[variant]


def _in_maps(key, query):
    _alpha, _beta, gt2, coff = _host_constants()
    s2 = key.reshape(B, J)
    h = J // 2
    maps = []
    for c in range(NCORES):
        b, half = divmod(c, 2)
        q = query[b].astype(np.float32)
        qhi = q.astype(np.float16)
        qlo = (q - qhi.astype(np.float32)).astype(np.float16)
        maps.append({
            "s": np.ascontiguousarray(
                s2[b, half * h:(half + 1) * h].reshape(P, F)),
            "qpair": np.ascontiguousarray(np.stack([qhi, qlo], 0)),
            "gt2": gt2,
            "coff": coff,
        })
    return maps


def kernel(key, query, _variant=None, _trace=False):
    key = np.ascontiguousarray(key, dtype=np.float32)
    query = np.ascontiguousarray(query, dtype=np.float32)
    nc = _get_nc(_variant or "fast")
    from concourse.bass_utils import run_bass_kernel_spmd

    res = run_bass_kernel_spmd(
        nc, _in_maps(key, query), list(range(NCORES)), trace=_trace)
    h = J // 2
    out = np.empty((B, J), np.float32)
    for c in range(NCORES):
        b, half = divmod(c, 2)
        out[b, half * h:(half + 1) * h] = res.results[c]["y"].reshape(h)
    if _trace:
        kernel.last_results = res
    return out.reshape(key.shape)


# revision 11
# speedup vs baseline: 2.8171x; 1.1613x over previous
"""Trainium2 Bass kernel for nn_AttentionMask_13048110645633.

Math: for key (4,32,64,64) and query (4,512), with s = key.reshape(B,J) and
q = query, the reference computes elementwise

    ctx[b,j] = sum_k q[b,k]*exp(s[b,j]*q[b,k]) / sum_k exp(s[b,j]*q[b,k])
    out[b,j] = s[b,j] * sigmoid(ctx[b,j])

i.e. out = s * g_b(s) with g_b a smooth scalar gate determined by q[b].
Sharding: data-parallel over B (4 batches x 2 half-slabs = 8 cores), one
(128,512) tile per core.

Device algorithm (per core):
  fit phase  - 128 fit nodes s_n synthesized on-device (iota+affine, no DMA);
               q broadcast to all partitions by a C=2 fp16-pair PE matmul;
               one ACT exp with per-partition scale + accumulate gives
               En=exp(s_n q) and S0; one DVE stt multiply-reduce gives
               0.5*S1; tanh(S1/(2*S0)) on ACT; the sigmoid affine and the
               least-squares fit both fold into a host fit matrix applied by
               two tiny PE matmuls -> even/odd polynomial coefficients of
               g in u = tanh(0.4 s), replicated on all 128 partitions.
  eval phase - u from one ACT tanh (fp16); v=u^2, v2=v^4 by DVE tt; the
               degree-7 polynomial evaluated Estrin-style with 4x-mode fp16
               tensor_scalar ops (per-partition coefficient pairs) and
               2x-mode fp16 tensor_tensor ops; final out = s*g in two fp32
               column halves.
  stores     - kv_writeback descriptors prepared on the SWDGE ring during
               the fit phase; each output half fires via trigger_dma as soon
               as its final multiply lands (no HWDGE issue latency on the
               tail).
"""

import os
import numpy as np

B, J, K = 4, 131072, 512
P, F = 128, 512   # per-core tile (P*F = J/2)
H = F // 2        # output store half-columns
NCORES = 8
NN = 128          # fit nodes (one per partition)
D = 6             # polynomial degree in u
WA = 0.4          # tanh warp: u = tanh(WA*s)
SR = 5.2          # node range: s_n uniform in (-SR, SR)
NE = D // 2 + 1   # even-part coeffs (poly in v=u^2)
NO = (D + 1) // 2 # odd-part coeffs
NC_ = NE + NO

_CONSTS = None
_NC_CACHE = {}


def _host_constants():
    """Data-independent fit constants: node affine map + folded fit matrix."""
    global _CONSTS
    if _CONSTS is not None:
        return _CONSTS
    alpha = 2.0 * SR / NN
    beta = -SR + SR / NN           # s_n = beta + alpha*n,  n = 0..NN-1
    n = np.arange(NN, dtype=np.float64)
    un = np.tanh(WA * (beta + alpha * n))
    vn = un * un
    Vb = np.concatenate([
        np.stack([vn**m for m in range(NE)], 1),
        np.stack([un * vn**m for m in range(NO)], 1)], 1)   # (NN, NC_)
    G = np.linalg.pinv(Vb)                                   # (NC_, NN)
    # gate = 0.5*tanh(0.5*ctx) + 0.5; the affine folds into the fit:
    # c = G @ gate = (0.5*G) @ tanhvals + G @ (0.5*ones)
    gt2 = np.ascontiguousarray((0.5 * G).T.astype(np.float16))   # (NN, NC_)
    coff = np.ascontiguousarray(
        (G @ (0.5 * np.ones(NN))).astype(np.float16).reshape(1, NC_))
    _CONSTS = (float(alpha), float(beta), gt2, coff)
    return _CONSTS


def _build_nc(variant):
    import concourse.bacc as bacc
    import concourse.bass as bass_mod
    import concourse.mybir as mybir
    from concourse import tile

    fp32 = mybir.dt.float32
    fp16 = mybir.dt.float16
    i32 = mybir.dt.int32
    AF = mybir.ActivationFunctionType
    OP = mybir.AluOpType
    alpha, beta, _gt2, _coff = _host_constants()

    nc = bacc.Bacc("TRN2", target_bir_lowering=False, debug=False,
                   num_devices=NCORES, num_swdge_queues=3)
    s_d = nc.dram_tensor("s", (P, F), fp32, kind="ExternalInput")
    qp_d = nc.dram_tensor("qpair", (2, K), fp16, kind="ExternalInput")
    gt_d = nc.dram_tensor("gt2", (NN, NC_), fp16, kind="ExternalInput")
    co_d = nc.dram_tensor("coff", (1, NC_), fp16, kind="ExternalInput")
    y_d = nc.dram_tensor("y", (P, F), fp32, kind="ExternalOutput")

    sem0 = nc.alloc_semaphore("wb0")
    sem1 = nc.alloc_semaphore("wb1")

    with tile.TileContext(nc) as tc:
        with (
            tc.tile_pool(name="c1", bufs=1) as cp,
            tc.tile_pool(name="ps", bufs=2, space="PSUM") as pp,
        ):
            # ---------------- Pool (gpsimd) queue: loads + metadata --------
            # bulk s tile via SWDGE (Pool prologue delay hides under the
            # longer q->En critical path); q goes first on SP-HWDGE
            s_all = cp.tile([P, F], fp32, tag="s_all")
            nc.gpsimd.dma_start(out=s_all[:], in_=s_d[:])
            zz = cp.tile([1, 1], fp32, tag="zz")
            nc.gpsimd.memset(zz[:], 0.0)
            io = cp.tile([P, 1], fp32, tag="io")
            nc.gpsimd.iota(io[:], [[1, 1]], channel_multiplier=1,
                           allow_small_or_imprecise_dtypes=True)
            ones2 = cp.tile([2, P], fp16, tag="ones2")
            nc.gpsimd.memset(ones2[:], 1.0)
            ones1 = cp.tile([1, P], fp16, tag="ones1")
            nc.gpsimd.memset(ones1[:], 1.0)
            cidx = cp.tile([P, 1], i32, tag="cidx")
            nc.gpsimd.memset(cidx[:], 0)

            # ---------------- SP queue: q + fit-matrix loads ---------------
            qp_sb = cp.tile([2, K], fp16, tag="qp_sb")
            nc.sync.dma_start(out=qp_sb[:], in_=qp_d[:])
            gtt = cp.tile([NN, NC_], fp16, tag="gtt")
            nc.sync.dma_start(out=gtt[:], in_=gt_d[:])
            coft = cp.tile([1, NC_], fp16, tag="coft")
            nc.sync.dma_start(out=coft[:], in_=co_d[:])

            # output tile + store descriptor prep (SWDGE ring, data deferred)
            outt = cp.tile([P, F], fp32, tag="outt")

            def wb(prep_q, sem, col0):
                oh = outt[:, col0:col0 + H]
                in4 = bass_mod.AP(oh.tensor, oh.offset,
                                  [list(oh.ap[0]), [F, 1], [F, 1],
                                   list(oh.ap[-1])])
                ya = y_d[:]
                out4 = bass_mod.AP(ya.tensor, ya.offset + col0,
                                   [[P * F, 1], [F, P], [F, 1], [1, H]])
                return nc.gpsimd.kv_writeback(
                    out4, in4, cidx[:],
                    prepare_only=True, sem=sem, queue_num=prep_q)

            # ---------------- ACT warmup: hoist the act-table load --------
            zz2 = cp.tile([1, 1], fp32, tag="zz2")
            nc.scalar.activation(zz2[:], zz[:], AF.Exp)

            # ---------------- fit-node pipeline ---------------------------
            snt = cp.tile([P, 1], fp32, tag="snt")
            nc.vector.tensor_scalar(out=snt[:], in0=io[:], scalar1=alpha,
                                    scalar2=beta, op0=OP.mult, op1=OP.add)

            q_ps = pp.tile([P, K], fp32, tag="qps")
            nc.tensor.matmul(q_ps[:], ones2[:], qp_sb[:], start=True, stop=True)

            En = cp.tile([NN, K], fp32, tag="En")
            S0n = cp.tile([NN, 1], fp32, tag="S0n")
            nc.scalar.activation(En[:], q_ps[:], AF.Exp, scale=snt[:],
                                 accum_out=S0n[:])
            # warp for the main tile; the zero bias rides on En so the ACT
            # queue schedules the node exp (coeff critical path) first
            zer = cp.tile([P, 1], fp32, tag="zer")
            nc.vector.tensor_scalar(out=zer[:], in0=En[:, 0:1], scalar1=0.0,
                                    scalar2=None, op0=OP.mult)
            T = cp.tile([P, F], fp16, tag="T")
            nc.scalar.activation(T[:], s_all[:], AF.Tanh, scale=float(WA),
                                 bias=zer[:])

            recn = cp.tile([NN, 1], fp32, tag="recn")
            nc.vector.reciprocal(recn[:], S0n[:])
            scr = cp.tile([NN, K], fp32, tag="scr")
            S1n = cp.tile([NN, 1], fp32, tag="S1n")
            nc.vector.scalar_tensor_tensor(
                out=scr[:], in0=En[:], scalar=0.5, in1=q_ps[:],
                op0=OP.mult, op1=OP.mult, accum_out=S1n[:])
            # thn = tanh(0.5*S1/S0) with the 0.5 folded into S1
            thn = cp.tile([NN, 1], fp32, tag="thn")
            nc.scalar.activation(thn[:], S1n[:], AF.Tanh, scale=recn[:])
            # broadcast along free dim for the PE contraction (fp16 lhsT)
            gbf = cp.tile([P, P], fp16, tag="gbf")
            nc.scalar.activation(gbf[:], En[:, 0:P], AF.Identity,
                                 bias=thn[:], scale=0.0)
            c_ps = pp.tile([P, NC_], fp32, tag="cps")
            nc.tensor.matmul(c_ps[:], gbf[:], gtt[:], start=True, stop=False)
            nc.tensor.matmul(c_ps[:], ones1[:], coft[:], start=False, stop=True)

            # ---------------- main evaluation (DVE, fp16) -----------------
            v = cp.tile([P, F], fp16, tag="v")
            nc.vector.tensor_tensor(v[:], T[:], T[:], OP.mult)
            v2 = cp.tile([P, F], fp16, tag="v2")
            nc.vector.tensor_tensor(v2[:], v[:], v[:], OP.mult)

            c_sb = cp.tile([P, NC_], fp32, tag="csb")
            nc.vector.tensor_copy(c_sb[:], c_ps[:])

            def col(i):
                return c_sb[:, i:i + 1]

            a = cp.tile([P, F], fp16, tag="a")
            nc.vector.tensor_scalar(out=a[:], in0=v[:], scalar1=col(1),
                                    scalar2=col(0), op0=OP.mult, op1=OP.add)
            bb = cp.tile([P, F], fp16, tag="bb")
            nc.vector.tensor_scalar(out=bb[:], in0=v[:], scalar1=col(3),
                                    scalar2=col(2), op0=OP.mult, op1=OP.add)
            t1 = cp.tile([P, F], fp16, tag="t1")
            nc.vector.tensor_tensor(t1[:], v2[:], bb[:], OP.mult)
            E = cp.tile([P, F], fp16, tag="E")
            nc.vector.tensor_tensor(E[:], t1[:], a[:], OP.add)
            cc = cp.tile([P, F], fp16, tag="cc")
            nc.vector.tensor_scalar(out=cc[:], in0=v[:], scalar1=col(NE + 1),
                                    scalar2=col(NE), op0=OP.mult, op1=OP.add)
            o2t = cp.tile([P, F], fp16, tag="o2t")
            nc.vector.tensor_scalar(out=o2t[:], in0=v2[:], scalar1=col(NE + 2),
                                    scalar2=None, op0=OP.mult)
            O = cp.tile([P, F], fp16, tag="O")
            nc.vector.tensor_tensor(O[:], o2t[:], cc[:], OP.add)
            t4 = cp.tile([P, F], fp16, tag="t4")
            nc.vector.tensor_tensor(t4[:], T[:], O[:], OP.mult)
            g = cp.tile([P, F], fp16, tag="g")
            nc.vector.tensor_tensor(g[:], E[:], t4[:], OP.add)

            # out = s*g in fp32 column halves; each half fires its store via
            # a pre-prepared SWDGE descriptor (the prep's data read defers to
            # the trigger, so the preps themselves run early on the idle ring
            # once _patch_store_sync moves their data waits onto the triggers)
            nc.vector.tensor_tensor(outt[:, 0:H], s_all[:, 0:H], g[:, 0:H],
                                    OP.mult)
            nc.vector.tensor_tensor(outt[:, H:F], s_all[:, H:F], g[:, H:F],
                                    OP.mult)
            wb(1, sem0, 0)
            wb(2, sem1, H)
            nc.gpsimd.trigger_dma(count=None, queue_num=1)
            nc.gpsimd.trigger_dma(count=None, queue_num=2)
            nc.gpsimd.wait_ge(sem0, 16)
            nc.gpsimd.wait_ge(sem1, 16)

    _patch_store_sync(nc, mybir)
    nc.compile()
    return nc


def _patch_store_sync(nc, mybir):
    """Post-schedule sync fixups for the triggered SWDGE stores.

    Tile places each writeback prep's (deferred) data wait BEFORE the prep,
    serializing the ~1us descriptor generation behind the final compute.  The
    prep only writes ring descriptors — its source read happens at trigger
    time — so the data wait belongs on the trigger.  Move it there.

    Tile's end-of-kernel drain also waits on the DMASW lane sems it assigned
    to the preps, but a prepare_only descriptor carries the caller's sem
    (wb0/wb1, which we wait on explicitly), so those lane sems never move.
    Strip waits on semaphores that no instruction updates.
    """
    fn = nc.m.functions[0]
    insts = [i for b in fn.blocks for i in b.instructions]
    pool = [i for i in insts if i.engine == mybir.EngineType.Pool]

    def waits(i):
        return list(i.sync_info.on_wait) if i.sync_info else []

    def ups(i):
        return list(i.sync_info.on_update) if i.sync_info else []

    def set_sync(i, w, u):
        i.sync_info = mybir.SyncInfo(on_wait=w, on_update=u)

    # move compute->prep waits onto the same-queue trigger
    for idx, ins in enumerate(pool):
        if type(ins).__name__ != "InstKVWritebackAnt":
            continue
        trig = next(t for t in pool[idx + 1:]
                    if type(t).__name__ == "InstTriggerDma"
                    and t.queue_num == ins.queue_num)
        moved = []
        for src in (pool[idx - 1], ins):
            if src is ins or type(src).__name__ == "InstEventSemaphore":
                keep = []
                for w in waits(src):
                    (moved if (w.ant_name or "").startswith("DVE")
                     else keep).append(w)
                if moved or keep != waits(src):
                    set_sync(src, keep, ups(src))
        if moved:
            set_sync(trig, waits(trig) + moved, ups(trig))

    # strip waits on semaphores nothing updates (dead lane sems)
    updated = {u.id for i in insts for u in ups(i)}
    for ins in insts:
        w = waits(ins)
        keep = [x for x in w if x.id in updated]
        if len(keep) != len(w):
            set_sync(ins, keep, ups(ins))


def _get_nc(variant="fast"):
    if variant not in _NC_CACHE:
        _NC_CACHE[variant] = _build_nc(variant)
    return _NC_CACHE[variant]


def _in_maps(key, query):
    _alpha, _beta, gt2, coff = _host_constants()
    s2 = key.reshape(B, J)
    h = J // 2
    maps = []
    for c in range(NCORES):
        b, half = divmod(c, 2)
        q = query[b].astype(np.float32)
        qhi = q.astype(np.float16)
        qlo = (q - qhi.astype(np.float32)).astype(np.float16)
        maps.append({
            "s": np.ascontiguousarray(
                s2[b, half * h:(half + 1) * h].reshape(P, F)),
            "qpair": np.ascontiguousarray(np.stack([qhi, qlo], 0)),
            "gt2": gt2,
            "coff": coff,
        })
    return maps


def kernel(key, query, _variant=None, _trace=False):
    key = np.ascontiguousarray(key, dtype=np.float32)
    query = np.ascontiguousarray(query, dtype=np.float32)
    nc = _get_nc(_variant or "fast")
    from concourse.bass_utils import run_bass_kernel_spmd

    res = run_bass_kernel_spmd(
        nc, _in_maps(key, query), list(range(NCORES)), trace=_trace)
    h = J // 2
    out = np.empty((B, J), np.float32)
    for c in range(NCORES):
        b, half = divmod(c, 2)
        out[b, half * h:(half + 1) * h] = res.results[c]["y"].reshape(h)
    if _trace:
        kernel.last_results = res
    return out.reshape(key.shape)


# revision 12
# speedup vs baseline: 2.9501x; 1.0472x over previous
"""Trainium2 Bass kernel for nn_AttentionMask_13048110645633.

Math: for key (4,32,64,64) and query (4,512), with s = key.reshape(B,J) and
q = query, the reference computes elementwise

    ctx[b,j] = sum_k q[b,k]*exp(s[b,j]*q[b,k]) / sum_k exp(s[b,j]*q[b,k])
    out[b,j] = s[b,j] * sigmoid(ctx[b,j])

i.e. out = s * g_b(s) with g_b a smooth scalar gate determined by q[b].
Sharding: data-parallel over B (4 batches x 2 half-slabs = 8 cores), one
(128,512) tile per core.

Device algorithm (per core):
  fit phase  - 128 fit nodes s_n synthesized on-device (iota+affine, no DMA);
               q broadcast to all partitions by a C=2 fp16-pair PE matmul;
               one ACT exp with per-partition scale + accumulate gives
               En=exp(s_n q) and S0; one DVE stt multiply-reduce gives
               0.5*S1; tanh(S1/(2*S0)) on ACT; the sigmoid affine and the
               least-squares fit both fold into a host fit matrix applied by
               two tiny PE matmuls -> even/odd polynomial coefficients of
               g in u = tanh(0.4 s), replicated on all 128 partitions.
  eval phase - u from one ACT tanh (fp16); v=u^2, v2=v^4 by DVE tt; the
               degree-7 polynomial evaluated Estrin-style with 4x-mode fp16
               tensor_scalar ops (per-partition coefficient pairs) and
               2x-mode fp16 tensor_tensor ops; final out = s*g in two fp32
               column halves.
  stores     - kv_writeback descriptors prepared on the SWDGE ring during
               the fit phase; each output half fires via trigger_dma as soon
               as its final multiply lands (no HWDGE issue latency on the
               tail).
"""

import os
import numpy as np

B, J, K = 4, 131072, 512
P, F = 128, 512   # per-core tile (P*F = J/2)
H = F // 2        # output store half-columns
NCORES = 8
NN = 128          # fit nodes (one per partition)
D = 5             # polynomial degree in u
WA = 0.5          # tanh warp: u = tanh(WA*s)
SR = 5.2          # node range: s_n uniform in (-SR, SR)
NE = D // 2 + 1   # even-part coeffs (poly in v=u^2)
NO = (D + 1) // 2 # odd-part coeffs
NC_ = NE + NO

_CONSTS = None
_NC_CACHE = {}


def _host_constants():
    """Data-independent fit constants: node affine map + folded fit matrix."""
    global _CONSTS
    if _CONSTS is not None:
        return _CONSTS
    alpha = 2.0 * SR / NN
    beta = -SR + SR / NN           # s_n = beta + alpha*n,  n = 0..NN-1
    n = np.arange(NN, dtype=np.float64)
    un = np.tanh(WA * (beta + alpha * n))
    vn = un * un
    Vb = np.concatenate([
        np.stack([vn**m for m in range(NE)], 1),
        np.stack([un * vn**m for m in range(NO)], 1)], 1)   # (NN, NC_)
    G = np.linalg.pinv(Vb)                                   # (NC_, NN)
    # gate = 0.5*tanh(0.5*ctx) + 0.5; the affine folds into the fit:
    # c = G @ gate = (0.5*G) @ tanhvals + G @ (0.5*ones)
    gt2 = np.ascontiguousarray((0.5 * G).T.astype(np.float16))   # (NN, NC_)
    coff = np.ascontiguousarray(
        (G @ (0.5 * np.ones(NN))).astype(np.float16).reshape(1, NC_))
    _CONSTS = (float(alpha), float(beta), gt2, coff)
    return _CONSTS


def _build_nc(variant):
    import concourse.bacc as bacc
    import concourse.bass as bass_mod
    import concourse.mybir as mybir
    from concourse import tile

    fp32 = mybir.dt.float32
    fp16 = mybir.dt.float16
    i32 = mybir.dt.int32
    AF = mybir.ActivationFunctionType
    OP = mybir.AluOpType
    alpha, beta, _gt2, _coff = _host_constants()

    nc = bacc.Bacc("TRN2", target_bir_lowering=False, debug=False,
                   num_devices=NCORES, num_swdge_queues=3)
    s_d = nc.dram_tensor("s", (P, F), fp32, kind="ExternalInput")
    qp_d = nc.dram_tensor("qpair", (2, K), fp16, kind="ExternalInput")
    gt_d = nc.dram_tensor("gt2", (NN, NC_), fp16, kind="ExternalInput")
    co_d = nc.dram_tensor("coff", (1, NC_), fp16, kind="ExternalInput")
    y_d = nc.dram_tensor("y", (P, F), fp32, kind="ExternalOutput")

    sem0 = nc.alloc_semaphore("wb0")
    sem1 = nc.alloc_semaphore("wb1")

    with tile.TileContext(nc) as tc:
        with (
            tc.tile_pool(name="c1", bufs=1) as cp,
            tc.tile_pool(name="ps", bufs=2, space="PSUM") as pp,
        ):
            # ---------------- Pool (gpsimd) queue: loads + metadata --------
            # bulk s tile via SWDGE (Pool prologue delay hides under the
            # longer q->En critical path); q goes first on SP-HWDGE
            s_all = cp.tile([P, F], fp32, tag="s_all")
            nc.gpsimd.dma_start(out=s_all[:], in_=s_d[:])
            zz = cp.tile([1, 1], fp32, tag="zz")
            nc.gpsimd.memset(zz[:], 0.0)
            io = cp.tile([P, 1], fp32, tag="io")
            nc.gpsimd.iota(io[:], [[1, 1]], channel_multiplier=1,
                           allow_small_or_imprecise_dtypes=True)
            ones2 = cp.tile([2, P], fp16, tag="ones2")
            nc.gpsimd.memset(ones2[:], 1.0)
            ones1 = cp.tile([1, P], fp16, tag="ones1")
            nc.gpsimd.memset(ones1[:], 1.0)
            cidx = cp.tile([P, 1], i32, tag="cidx")
            nc.gpsimd.memset(cidx[:], 0)

            # ---------------- SP queue: q + fit-matrix loads ---------------
            qp_sb = cp.tile([2, K], fp16, tag="qp_sb")
            nc.sync.dma_start(out=qp_sb[:], in_=qp_d[:])
            gtt = cp.tile([NN, NC_], fp16, tag="gtt")
            nc.sync.dma_start(out=gtt[:], in_=gt_d[:])
            coft = cp.tile([1, NC_], fp16, tag="coft")
            nc.sync.dma_start(out=coft[:], in_=co_d[:])

            # output tile + store descriptor prep (SWDGE ring, data deferred)
            outt = cp.tile([P, F], fp32, tag="outt")

            def wb(prep_q, sem, col0):
                oh = outt[:, col0:col0 + H]
                in4 = bass_mod.AP(oh.tensor, oh.offset,
                                  [list(oh.ap[0]), [F, 1], [F, 1],
                                   list(oh.ap[-1])])
                ya = y_d[:]
                out4 = bass_mod.AP(ya.tensor, ya.offset + col0,
                                   [[P * F, 1], [F, P], [F, 1], [1, H]])
                return nc.gpsimd.kv_writeback(
                    out4, in4, cidx[:],
                    prepare_only=True, sem=sem, queue_num=prep_q)

            # ---------------- ACT warmup: hoist the act-table load --------
            zz2 = cp.tile([1, 1], fp32, tag="zz2")
            nc.scalar.activation(zz2[:], zz[:], AF.Exp)

            # ---------------- fit-node pipeline ---------------------------
            snt = cp.tile([P, 1], fp32, tag="snt")
            nc.vector.tensor_scalar(out=snt[:], in0=io[:], scalar1=alpha,
                                    scalar2=beta, op0=OP.mult, op1=OP.add)

            q_ps = pp.tile([P, K], fp32, tag="qps")
            nc.tensor.matmul(q_ps[:], ones2[:], qp_sb[:], start=True, stop=True)

            En = cp.tile([NN, K], fp32, tag="En")
            S0n = cp.tile([NN, 1], fp32, tag="S0n")
            nc.scalar.activation(En[:], q_ps[:], AF.Exp, scale=snt[:],
                                 accum_out=S0n[:])
            # warp for the main tile; the zero bias rides on En so the ACT
            # queue schedules the node exp (coeff critical path) first
            zer = cp.tile([P, 1], fp32, tag="zer")
            nc.vector.tensor_scalar(out=zer[:], in0=En[:, 0:1], scalar1=0.0,
                                    scalar2=None, op0=OP.mult)
            T = cp.tile([P, F], fp16, tag="T")
            nc.scalar.activation(T[:], s_all[:], AF.Tanh, scale=float(WA),
                                 bias=zer[:])

            recn = cp.tile([NN, 1], fp32, tag="recn")
            nc.vector.reciprocal(recn[:], S0n[:])
            scr = cp.tile([NN, K], fp32, tag="scr")
            S1n = cp.tile([NN, 1], fp32, tag="S1n")
            nc.vector.scalar_tensor_tensor(
                out=scr[:], in0=En[:], scalar=0.5, in1=q_ps[:],
                op0=OP.mult, op1=OP.mult, accum_out=S1n[:])
            # thn = tanh(0.5*S1/S0) with the 0.5 folded into S1; fp16 so it
            # feeds the PE contraction directly via a stride-0 broadcast AP
            thn = cp.tile([NN, 1], fp16, tag="thn")
            nc.scalar.activation(thn[:], S1n[:], AF.Tanh, scale=recn[:])
            c_ps = pp.tile([P, NC_], fp32, tag="cps")
            nc.tensor.matmul(c_ps[:], thn[:].broadcast_to([NN, P]), gtt[:],
                             start=True, stop=False)
            nc.tensor.matmul(c_ps[:], ones1[:], coft[:], start=False, stop=True)

            # ---------------- main evaluation (DVE, fp16) -----------------
            v = cp.tile([P, F], fp16, tag="v")
            nc.vector.tensor_tensor(v[:], T[:], T[:], OP.mult)
            v2 = cp.tile([P, F], fp16, tag="v2")
            nc.vector.tensor_tensor(v2[:], v[:], v[:], OP.mult)

            c_sb = cp.tile([P, NC_], fp32, tag="csb")
            nc.vector.tensor_copy(c_sb[:], c_ps[:])

            def col(i):
                return c_sb[:, i:i + 1]

            a = cp.tile([P, F], fp16, tag="a")
            nc.vector.tensor_scalar(out=a[:], in0=v[:], scalar1=col(1),
                                    scalar2=col(0), op0=OP.mult, op1=OP.add)
            e2t = cp.tile([P, F], fp16, tag="e2t")
            nc.vector.tensor_scalar(out=e2t[:], in0=v2[:], scalar1=col(2),
                                    scalar2=None, op0=OP.mult)
            E = cp.tile([P, F], fp16, tag="E")
            nc.vector.tensor_tensor(E[:], e2t[:], a[:], OP.add)
            cc = cp.tile([P, F], fp16, tag="cc")
            nc.vector.tensor_scalar(out=cc[:], in0=v[:], scalar1=col(NE + 1),
                                    scalar2=col(NE), op0=OP.mult, op1=OP.add)
            o2t = cp.tile([P, F], fp16, tag="o2t")
            nc.vector.tensor_scalar(out=o2t[:], in0=v2[:], scalar1=col(NE + 2),
                                    scalar2=None, op0=OP.mult)
            O = cp.tile([P, F], fp16, tag="O")
            nc.vector.tensor_tensor(O[:], o2t[:], cc[:], OP.add)
            t4 = cp.tile([P, F], fp16, tag="t4")
            nc.vector.tensor_tensor(t4[:], T[:], O[:], OP.mult)
            g = cp.tile([P, F], fp16, tag="g")
            nc.vector.tensor_tensor(g[:], E[:], t4[:], OP.add)

            # out = s*g in fp32 column halves; each half fires its store via
            # a pre-prepared SWDGE descriptor (the prep's data read defers to
            # the trigger, so the preps themselves run early on the idle ring
            # once _patch_store_sync moves their data waits onto the triggers)
            nc.vector.tensor_tensor(outt[:, 0:H], s_all[:, 0:H], g[:, 0:H],
                                    OP.mult)
            nc.vector.tensor_tensor(outt[:, H:F], s_all[:, H:F], g[:, H:F],
                                    OP.mult)
            wb(1, sem0, 0)
            wb(2, sem1, H)
            nc.gpsimd.trigger_dma(count=None, queue_num=1)
            nc.gpsimd.trigger_dma(count=None, queue_num=2)
            nc.gpsimd.wait_ge(sem0, 16)
            nc.gpsimd.wait_ge(sem1, 16)

    _patch_store_sync(nc, mybir)
    nc.compile()
    return nc


def _patch_store_sync(nc, mybir):
    """Post-schedule sync fixups for the triggered SWDGE stores.

    Tile places each writeback prep's (deferred) data wait BEFORE the prep,
    serializing the ~1us descriptor generation behind the final compute.  The
    prep only writes ring descriptors — its source read happens at trigger
    time — so the data wait belongs on the trigger.  Move it there.

    Tile's end-of-kernel drain also waits on the DMASW lane sems it assigned
    to the preps, but a prepare_only descriptor carries the caller's sem
    (wb0/wb1, which we wait on explicitly), so those lane sems never move.
    Strip waits on semaphores that no instruction updates.
    """
    fn = nc.m.functions[0]
    insts = [i for b in fn.blocks for i in b.instructions]
    pool = [i for i in insts if i.engine == mybir.EngineType.Pool]

    def waits(i):
        return list(i.sync_info.on_wait) if i.sync_info else []

    def ups(i):
        return list(i.sync_info.on_update) if i.sync_info else []

    def set_sync(i, w, u):
        i.sync_info = mybir.SyncInfo(on_wait=w, on_update=u)

    # move compute->prep waits onto the same-queue trigger
    for idx, ins in enumerate(pool):
        if type(ins).__name__ != "InstKVWritebackAnt":
            continue
        trig = next(t for t in pool[idx + 1:]
                    if type(t).__name__ == "InstTriggerDma"
                    and t.queue_num == ins.queue_num)
        moved = []
        for src in (pool[idx - 1], ins):
            if src is ins or type(src).__name__ == "InstEventSemaphore":
                keep = []
                for w in waits(src):
                    (moved if (w.ant_name or "").startswith("DVE")
                     else keep).append(w)
                if moved or keep != waits(src):
                    set_sync(src, keep, ups(src))
        if moved:
            set_sync(trig, waits(trig) + moved, ups(trig))

    # strip waits on semaphores nothing updates (dead lane sems)
    updated = {u.id for i in insts for u in ups(i)}
    for ins in insts:
        w = waits(ins)
        keep = [x for x in w if x.id in updated]
        if len(keep) != len(w):
            set_sync(ins, keep, ups(ins))


def _get_nc(variant="fast"):
    if variant not in _NC_CACHE:
        _NC_CACHE[variant] = _build_nc(variant)
    return _NC_CACHE[variant]


def _in_maps(key, query):
    _alpha, _beta, gt2, coff = _host_constants()
    s2 = key.reshape(B, J)
    h = J // 2
    maps = []
    for c in range(NCORES):
        b, half = divmod(c, 2)
        q = query[b].astype(np.float32)
        qhi = q.astype(np.float16)
        qlo = (q - qhi.astype(np.float32)).astype(np.float16)
        maps.append({
            "s": np.ascontiguousarray(
                s2[b, half * h:(half + 1) * h].reshape(P, F)),
            "qpair": np.ascontiguousarray(np.stack([qhi, qlo], 0)),
            "gt2": gt2,
            "coff": coff,
        })
    return maps


def kernel(key, query, _variant=None, _trace=False):
    key = np.ascontiguousarray(key, dtype=np.float32)
    query = np.ascontiguousarray(query, dtype=np.float32)
    nc = _get_nc(_variant or "fast")
    from concourse.bass_utils import run_bass_kernel_spmd

    res = run_bass_kernel_spmd(
        nc, _in_maps(key, query), list(range(NCORES)), trace=_trace)
    h = J // 2
    out = np.empty((B, J), np.float32)
    for c in range(NCORES):
        b, half = divmod(c, 2)
        out[b, half * h:(half + 1) * h] = res.results[c]["y"].reshape(h)
    if _trace:
        kernel.last_results = res
    return out.reshape(key.shape)
